# revision 71
# baseline (speedup 1.0000x reference)
"""Trainium2 Bass kernel for nn_DecLayer (GNN message-passing decoder layer).

Strategy
--------
Data-parallel over nodes: 10000 nodes are padded and split 1280 per core
across 8 NeuronCores.  Per core everything is computed in "transposed space"
(feature dim on SBUF partitions, edges/nodes on the free dim).

Key optimizations over the v1 kernel:
  * h_E is pre-quantized to fp8(e4m3) on the host -> HBM traffic for the
    dominant stream drops 4x vs f32 (15.7MB/core, ~44us at modeled BW).
  * All edge-level matmuls run in fp8 DoubleRow perf mode (2 contraction
    tiles per pass, 0.5 cycles/row).  Weights are pre-scaled by powers of
    two into the fp8 normal range; the activation instruction's `scale`
    operand undoes the scaling for free.  Odd contraction tiles are padded
    by reading a data tile twice against zero weights (stride-0 AP).
  * The K=32 neighbor sum is folded into the W3 matmul: 32 accumulating
    PSUM matmuls with k-strided moving APs replace the (slow, no-perf-mode)
    DVE TensorReduce entirely.
  * LayerNorm stats/broadcasts use bf16 ones-matmuls (1 cycle/row instead
    of 4 for f32).
  * gelu instructions cover 1024 edges (2 PSUM banks) to amortize ACT
    SBUF-access overhead; ScalarE is the bottleneck engine at ~93us busy.
    Individual matmuls stay within one PSUM bank (512 f32) as the ISA
    requires; the 2-bank activation reads span both halves.
  * Group tails (W3/LN/FFN/LN2/store) are split into parts and interleaved
    into the supertile pipeline (2 parts per supertile); the last group's
    tail runs as staggered pieces, the final two half-width chains zipped.
    The latency-exposed final pieces use f32 stats matmuls (no bf16 copy
    hop) and f32 row constants.  PSUM-reading elementwise ops always run
    on DVE (GPSIMD has no PSUM access).
  * The K=32 neighbor sum pairs neighbors k/k+1 per DoubleRow matmul
    (16 matmuls per 256-node piece).
  * Startup: the first 1024-edge h_E chunk + packed W1 go out first on
    the SP/ACT HWDGE queues (one DMA each); cold constants are merged
    into 5 composite DMAs so the SP issue queue stays short; a PE warmup
    matmul chain starts the p-state ramp at t~0.  The SWDGE bulk stream
    opens with a tiny chunk so its second DMA lands after the startup
    critical loads.

Modeled per-core time: ~112.8us (vs 119us for the previous fp8 kernel and
180us for the f32/bf16 v1); measured end-to-end hardware rel-err vs the
f32 reference: 4.7e-3.

If the runtime masks are all ones (true for this problem's setup_inputs),
the kernel compiles without any mask handling; a general path handles
arbitrary masks.
"""

import json

import numpy as np
import ml_dtypes

import concourse.bass as bass
import concourse.mybir as mybir
import concourse.tile as tile

F32 = mybir.dt.float32
BF16 = mybir.dt.bfloat16
FP8 = mybir.dt.float8e4
AF = mybir.ActivationFunctionType
OP = mybir.AluOpType
AX = mybir.AxisListType
PM = mybir.MatmulPerfMode

E4 = ml_dtypes.float8_e4m3

H = 128        # hidden
C = 384        # edge feature dim
K = 32         # neighbors
N_CORES = 8
GROUP = 256                      # nodes per group
ST_NODES = 32                    # nodes per supertile
ST_E = ST_NODES * K              # 1024 edges per supertile
SCALE = 30.0
EPS = 1e-5
S1 = 16.0                        # W1 fp8 pre-scale
S2 = 8.0                         # W2 fp8 pre-scale
S3 = 256.0 / SCALE               # W3 fp8 pre-scale (folds 1/SCALE)


# ---------------------------------------------------------------------------
# walrus workaround: this build rejects >1 sync wait per instruction; split
# extra waits into standalone EventSemaphore instructions on the same engine
# (engines execute their stream in order, so semantics are preserved).
# ---------------------------------------------------------------------------
def _split_multi_waits(bir_json: bytes) -> bytes:
    m = json.loads(bir_json)
    for f in m.get("functions", []):
        for b in f.get("blocks", []):
            out = []
            for inst in b.get("instructions", []):
                si = inst.get("sync_info")
                waits = (si or {}).get("on_wait") or []
                if len(waits) > 1:
                    for j, w in enumerate(waits[:-1]):
                        out.append({
                            "debug": inst.get("debug", 0),
                            "engine": inst["engine"],
                            "ins": [], "outs": [],
                            "name": f"{inst['name']}_wsplit{j}",
                            "opcode": "EventSemaphore",
                            "sync_info": {"on_update": [], "on_wait": [w]},
                        })
                    si["on_wait"] = waits[-1:]
                out.append(inst)
            b["instructions"] = out
    return json.dumps(m).encode()


def _install_bir_fix():
    import concourse.bass_utils as bu
    import concourse.bass2jax as b2j
    if getattr(bu, "_wsplit_installed", False):
        return
    orig = bu.compile_bir_kernel

    def patched(bir_json, tmpdir, neff_name="file.neff"):
        return orig(_split_multi_waits(bir_json), tmpdir, neff_name)

    bu.compile_bir_kernel = patched
    b2j.compile_bir_kernel = patched
    bu._wsplit_installed = True


def _bf(x):
    return np.asarray(x, dtype=ml_dtypes.bfloat16)


def _f8(x):
    return np.asarray(x, dtype=E4)


def _dr_pair(a, b):
    """Stack two [128, M] weight blocks into DoubleRow lhsT layout
    [128, 2*M] = [p, (t m)]."""
    return np.stack([a, b], axis=1).reshape(128, -1)


def build_nc(w, n_nodes, ones_masks=False):
    """Build the per-core Bass kernel. `w` holds the (host numpy) weights."""
    assert n_nodes % GROUP == 0
    n_groups = n_nodes // GROUP
    n_edges = n_nodes * K
    n_st = GROUP // ST_NODES

    nc = bass.Bass()

    hE8 = nc.dram_tensor("hE8", [C, n_edges], FP8, kind="ExternalInput")
    hv8D = nc.dram_tensor("hv8D", [H, n_nodes], FP8, kind="ExternalInput")
    hVt = nc.dram_tensor("hVt", [H, n_nodes], F32, kind="ExternalInput")
    maskr = nc.dram_tensor("maskr", [1, n_edges], BF16, kind="ExternalInput")
    msumb = nc.dram_tensor("msumb", [1, n_nodes], BF16, kind="ExternalInput")
    maskVr = nc.dram_tensor("maskVr", [1, n_nodes], BF16, kind="ExternalInput")
    outt = nc.dram_tensor("outt", [H, n_nodes], F32, kind="ExternalOutput")

    # --- inline constants (weights known at build time) ---
    W1 = w["W1_w"]          # [H+C, H]
    zero8 = np.zeros((128, H), E4)
    w1a_h = _dr_pair(_f8(W1[H:H + 128] * S1), _f8(W1[H + 128:H + 256] * S1))
    w1b_h = _dr_pair(_f8(W1[H + 256:H + 384] * S1), zero8)
    w1v_h = _dr_pair(_f8(W1[:H] * S1), zero8)
    w2d_h = _dr_pair(_f8(w["W2_w"] * S2), zero8)
    w3d_h = _dr_pair(_f8(w["W3_w"] * S3), zero8)
    w3b_h = _bf(w["W3_w"] / SCALE)                      # general-mask path
    wout_h = np.concatenate(
        [w["Wout_w"][128 * j: 128 * (j + 1), :] for j in range(4)], axis=1)

    # hot W1 blocks packed into one tensor -> a single startup DMA
    w1all_h = np.concatenate([w1a_h, w1b_h, w1v_h], axis=1)   # [128, 3*256]
    w1allD = nc.inline_tensor(w1all_h, name="w1all")
    if ones_masks:
        # both DoubleRow tiles carry the real weights: the K-sum pairs
        # neighbors k/k+1 per matmul (16 matmuls instead of 32).
        # w2d and w3d are packed into one inline tensor / one DMA.
        w3d_h = _dr_pair(_f8(w["W3_w"] * S3), _f8(w["W3_w"] * S3))
        w23dD = nc.inline_tensor(
            np.concatenate([w2d_h, w3d_h], axis=1), name="w23d")
    else:
        w2dD = nc.inline_tensor(w2d_h, name="w2d")
        w3bD = nc.inline_tensor(w3b_h, name="w3b")
        b3sD = nc.inline_tensor(_bf(w["W3_b"] / SCALE).reshape(1, H), name="b3s")
    # merged cold constants: fewer DMA issues on the SP sequencer
    winoutD = nc.inline_tensor(
        np.concatenate([_bf(w["Win_w"]), _bf(wout_h)], axis=1), name="winout")
    onescD = nc.inline_tensor(np.ones((H, 1), ml_dtypes.bfloat16), name="onesc")
    # row constants [1, X] packed: onesr(H) | onesg(GROUP) | woutb(H) |
    # g1row(H) | g2row(H)
    rows_h = np.concatenate([
        np.ones((1, H), ml_dtypes.bfloat16),
        np.ones((1, GROUP), ml_dtypes.bfloat16),
        _bf(w["Wout_b"]).reshape(1, H),
        _bf(w["ln1_g"]).reshape(1, H),
        _bf(w["ln2_g"]).reshape(1, H),
    ], axis=1)
    rowsD = nc.inline_tensor(rows_h, name="rows")
    # per-partition bias columns: b1, b2, ln1b, ln2b, winb0..3, EPS, ones
    cols_h = np.stack(
        [w["W1_b"], w["W2_b"], w["ln1_b"], w["ln2_b"]]
        + [w["Win_b"][128 * j: 128 * (j + 1)] for j in range(4)]
        + [np.full(H, EPS, np.float32), np.ones(H, np.float32)],
        axis=1,
    ).astype(np.float32)
    colsD = nc.inline_tensor(cols_h, name="cols")
    # f32 row constants for the latency-critical final pieces:
    # [ones; ln1_g; ln2_g]
    rows32_h = np.concatenate(
        [np.ones(H, np.float32), w["ln1_g"].astype(np.float32),
         w["ln2_g"].astype(np.float32)]).reshape(1, 3 * H)
    rows32D = nc.inline_tensor(rows32_h, name="rows32")

    # value specializations (checked against actual weights at build time)
    b1z = not np.any(w["W1_b"])
    b2z = not np.any(w["W2_b"])
    b3z = not np.any(w["W3_b"])
    winbz = not np.any(w["Win_b"])
    woutbz = not np.any(w["Wout_b"])
    ln1bz = not np.any(w["ln1_b"])
    ln2bz = not np.any(w["ln2_b"])

    g2_dt = FP8 if ones_masks else BF16

    with tile.TileContext(nc) as tc:
        with (
            tc.tile_pool(name="const", bufs=1) as constp,
            tc.tile_pool(name="xe", bufs=3) as xep,
            tc.tile_pool(name="g1p", bufs=3) as g1p,
            tc.tile_pool(name="g2p", bufs=3) as g2p,
            tc.tile_pool(name="st", bufs=3) as stp,
            tc.tile_pool(name="grp", bufs=3) as grpp,
            tc.tile_pool(name="ph", bufs=3 if ones_masks else 2,
                         space="PSUM") as php,
            tc.tile_pool(name="pm", bufs=1, space="PSUM") as pmp,
            tc.tile_pool(name="gp", bufs=2, space="PSUM") as pgr,
        ):
            # ---- load constants once (SP/HWDGE; bulk streams go via Pool) ----
            def cload(shape, dt_, src, name):
                t = constp.tile(shape, dt_, name=name)
                nc.sync.dma_start(t[:], src)
                return t

            # PE warm-up: a chain of dummy matmuls keeps the Tensor engine
            # busy from t~0 until the first real W1 data lands (~4.4us), so
            # the p-state ramp reaches full frequency before real work.
            wupb = constp.tile([128, 1], BF16, name="wup")
            nc.vector.memset(wupb[:], 0.0)
            pwu = pgr.tile([1, 512], F32, tag="gp", name="pwu")
            for _ in range(6):
                nc.tensor.matmul(pwu[:], wupb[:],
                                 wupb[:].broadcast_to([128, 512]))

            # hot constants (single packed DMA, emitted from group_loads(0)
            # right after the critical first h_E chunk)
            hot = {}

            def emit_hot_consts():
                # issued from the ACT HWDGE queue (idle until the first gelu)
                # so it doesn't wait behind the first h_E chunk in the SP
                # sequencer
                w1all_s = constp.tile([128, 3, 2, H], FP8, name="w1all_s")
                nc.scalar.dma_start(
                    w1all_s[:],
                    w1allD[:].rearrange("p (j t m) -> p j t m", j=3, t=2))
                hot["w1a"], hot["w1b"], hot["w1v"] = (
                    w1all_s[:, 0], w1all_s[:, 1], w1all_s[:, 2])

            def cold_consts():
                """Emitted after group-0 loads: first used by W2 / the tails.
                Merged into 5 DMAs to keep the SP issue queue short."""
                c = {}
                if ones_masks:
                    w23 = constp.tile([128, 2, 2, H], FP8, name="w23_s")
                    nc.sync.dma_start(
                        w23[:],
                        w23dD[:].rearrange("p (a t m) -> p a t m", a=2, t=2))
                    c["w2d_s"] = w23[:, 0]
                    c["w3d_s"] = w23[:, 1]
                else:
                    c["w2d_s"] = cload(
                        [128, 2, H], FP8,
                        w2dD[:].rearrange("p (t m) -> p t m", t=2), "w2d_s")
                    c["w3b_s"] = cload([128, H], BF16, w3bD[:], "w3b_s")
                    c["b3s_s"] = cload([1, H], BF16, b3sD[:], "b3s_s")
                wio = constp.tile([H, 2, 512], BF16, name="winout_s")
                nc.sync.dma_start(
                    wio[:], winoutD[:].rearrange("p (a m) -> p a m", a=2))
                c["win_s"] = wio[:, 0]
                c["wout_s"] = wio[:, 1]
                rows = constp.tile([1, rows_h.shape[1]], BF16, name="rows_s")
                nc.sync.dma_start(rows[:], rowsD[:])
                c["onesr_s"] = rows[:, 0:H]
                c["onesg_s"] = rows[:, H:H + GROUP]
                c["woutb_s"] = rows[:, H + GROUP:2 * H + GROUP]
                c["g1row_s"] = rows[:, 2 * H + GROUP:3 * H + GROUP]
                c["g2row_s"] = rows[:, 3 * H + GROUP:4 * H + GROUP]
                c["cols_s"] = cload([H, cols_h.shape[1]], F32, colsD[:], "cols_s")
                c["onesc_s"] = cload([H, 1], BF16, onescD[:], "onesc_s")
                c["rows32_s"] = cload([1, 3 * H], F32, rows32D[:], "rows32_s")
                return c

            cc = {}

            def col(i):
                return cc["cols_s"][:, i:i + 1]

            # ---------------- per-group state ----------------
            gstate = {}   # g -> dict of tiles
            ststate = {}  # t -> dict of tiles
            mstate = {}

            def group_loads(g):
                e0 = g * GROUP * K
                n0 = g * GROUP
                st = {}
                st["xe"] = xep.tile([H, 3, GROUP * K], FP8, tag="xe", name="xe")
                hEr = hE8[:].rearrange("(j p) e -> p j e", p=128)
                if g == 0:
                    # startup critical path, all on the fast HWDGE/SP queue:
                    # first 1024-edge chunk -> packed W1 -> hv8; the rest of
                    # the group streams via gpsimd SWDGE.
                    emit_hot_consts()
                    nc.sync.dma_start(st["xe"][:, :, 0:1024],
                                      hEr[:, :, e0:e0 + 1024])
                    # tiny first SWDGE chunk: its DMA grab is ~0.2us, and the
                    # next chunk's descriptor-gen delays the Pool stream's
                    # second DMA past the startup critical loads
                    bounds = [1024, 1056, 2048, 3072, 4096, 6144, 8192]
                else:
                    bounds = [0, 2048, 4096, 8192]
                st["hv8"] = grpp.tile([H, GROUP], FP8, tag="hv8", name="hv8")
                nc.sync.dma_start(st["hv8"][:], hv8D[:, n0:n0 + GROUP])
                st["hvf"] = grpp.tile([H, GROUP], F32, tag="hvf", name="hvf")
                nc.sync.dma_start(st["hvf"][:], hVt[:, n0:n0 + GROUP])
                if not ones_masks:
                    st["mV"] = grpp.tile([1, GROUP], BF16, tag="mV", name="mV")
                    nc.sync.dma_start(st["mV"][:], maskVr[:, n0:n0 + GROUP])
                    if not b3z:
                        st["msum"] = grpp.tile([1, GROUP], BF16, tag="msum", name="msum")
                        nc.sync.dma_start(st["msum"][:], msumb[:, n0:n0 + GROUP])
                # bulk h_E stream: chunked via gpsimd SWDGE so the HWDGE/SP
                # queue stays free for small loads and stores.
                for hh in range(len(bounds) - 1):
                    nc.gpsimd.dma_start(
                        st["xe"][:, :, bounds[hh]:bounds[hh + 1]],
                        hEr[:, :, e0 + bounds[hh]:e0 + bounds[hh + 1]],
                    )
                st["g2"] = g2p.tile([H, GROUP * K], g2_dt, tag="g2", name="g2")
                gstate[g] = st

            def mrow_load(t):
                if ones_masks or t >= n_groups * n_st:
                    return
                mrow = stp.tile([1, ST_E], BF16, tag="mrow", name="mrow")
                nc.sync.dma_start(mrow[:], maskr[:, t * ST_E:(t + 1) * ST_E])
                mstate[t] = mrow

            def stage_A(t):
                """W1 matmuls for supertile t -> ph (2 PSUM banks)."""
                g, s = divmod(t, n_st)
                gs = gstate[g]
                c0 = s * ST_E
                st = {}
                ph1 = php.tile([H, ST_E], F32, tag="ph", name="ph1")
                HB = ST_E // 2          # matmul out must stay in one PSUM bank
                HN = ST_NODES // 2
                for h in range(2):
                    dst = ph1[:, h * HB:(h + 1) * HB]
                    e0h = c0 + h * HB
                    nc.tensor.matmul(dst, hot["w1a"], gs["xe"][:, 0:2, e0h:e0h + HB],
                                     start=True, stop=False, perf_mode=PM.DoubleRow)
                    nc.tensor.matmul(
                        dst, hot["w1b"],
                        gs["xe"][:, 2:3, e0h:e0h + HB].broadcast_to([H, 2, HB]),
                        start=False, stop=False, perf_mode=PM.DoubleRow)
                    n0h = s * ST_NODES + h * HN
                    hv_b = (gs["hv8"][:, n0h:n0h + HN]
                            .rearrange("p (o n) -> p o n", o=1)
                            .broadcast_to([H, 2, HN])
                            .broadcast_to([H, 2, HN, K]))
                    nc.tensor.matmul(dst, hot["w1v"], hv_b,
                                     start=False, stop=True, perf_mode=PM.DoubleRow)
                st["ph1"] = ph1
                if not ones_masks:
                    pm = pmp.tile([H, ST_E], F32, tag="pm", name="pm")
                    mrow = mstate.pop(t)
                    for h in range(2):
                        nc.tensor.matmul(pm[:, h * (ST_E // 2):(h + 1) * (ST_E // 2)],
                                         cc["onesr_s"][:],
                                         mrow[:, h * (ST_E // 2):(h + 1) * (ST_E // 2)])
                    st["pm"] = pm
                ststate[t] = st

            def stage_B(t):
                """gelu1 over 1024 edges; un-scales W1 by 1/S1; fp8 out."""
                st = ststate[t]
                g1 = g1p.tile([H, 1, ST_E], FP8, tag="g1", name="g1")
                nc.scalar.activation(g1[:, 0, :], st["ph1"][:], AF.Gelu,
                                     bias=0.0 if b1z else col(0), scale=1.0 / S1)
                st["g1"] = g1

            def stage_C(t):
                """W2 fp8 DoubleRow (dup-tile) -> ph2."""
                st = ststate[t]
                ph2 = php.tile([H, ST_E], F32, tag="ph", name="ph2")
                HB = ST_E // 2
                for h in range(2):
                    nc.tensor.matmul(
                        ph2[:, h * HB:(h + 1) * HB], cc["w2d_s"][:],
                        st["g1"][:, 0:1, h * HB:(h + 1) * HB]
                        .broadcast_to([H, 2, HB]),
                        start=True, stop=True, perf_mode=PM.DoubleRow)
                st["ph2"] = ph2

            def stage_D(t):
                """gelu2 -> slice of the group g2 tile (fp8 or bf16)."""
                g, s = divmod(t, n_st)
                st = ststate.pop(t)
                gs = gstate[g]
                dst = gs["g2"][:, s * ST_E:(s + 1) * ST_E]
                if ones_masks:
                    nc.scalar.activation(dst, st["ph2"][:], AF.Gelu,
                                         bias=0.0 if b2z else col(1), scale=1.0 / S2)
                else:
                    g2t = stp.tile([H, ST_E], BF16, tag="g2t", name="g2t")
                    nc.scalar.activation(g2t[:], st["ph2"][:], AF.Gelu,
                                         bias=0.0 if b2z else col(1), scale=1.0 / S2)
                    nc.vector.tensor_tensor(dst, g2t[:], st["pm"][:], op=OP.mult)

            # ---------------- group tails ----------------
            def ln_block(src_f32, src_bf, sq_bf, gamma_row, beta_i, beta_z, nm, W,
                         ptile, ve, fin=False, gidx=1):
                """Shared LN piece: stats matmuls -> mean/rstd -> normalized.

                src_f32: [H, GROUP] f32 (the residual input)
                src_bf/sq_bf: bf16 copies (src, src^2) for the stats matmuls
                fin: latency-critical final piece — f32 stats matmuls read the
                residual directly (no bf16 copy hop); gidx selects the f32
                gamma row (1=ln1, 2=ln2).
                Returns list of part-functions; leaves result tile in ctx[nm].
                """
                ctx = {}
                stat_dt = F32 if fin else BF16

                def s1():
                    onesc = (cc["cols_s"][:, 9:10] if fin
                             else cc["onesc_s"][:])
                    ps1 = pgr.tile([1, W], F32, tag="gp", name=f"ps1{nm}")
                    nc.tensor.matmul(ps1[:], onesc, src_bf())
                    ps2 = pgr.tile([1, W], F32, tag="gp", name=f"ps2{nm}")
                    nc.tensor.matmul(ps2[:], onesc, sq_bf())
                    ctx["ps1"], ctx["ps2"] = ps1, ps2

                def s2():
                    mean = grpp.tile([1, W], stat_dt, tag=f"mean{nm}", name=f"mean{nm}")
                    tmp = grpp.tile([1, W], F32, tag=f"m2{nm}", name=f"m2{nm}")
                    varh = grpp.tile([1, W], F32, tag=f"var{nm}", name=f"var{nm}")
                    sd = grpp.tile([1, W], F32, tag=f"sd{nm}", name=f"sd{nm}")
                    rstd = grpp.tile([1, W], stat_dt, tag=f"rstd{nm}", name=f"rstd{nm}")
                    with nc.allow_low_precision("bf16 LN stats validated vs ref"):
                        # PSUM operands must stay off GPSIMD (no PSUM access)
                        nc.vector.tensor_scalar_mul(mean[:], ctx["ps1"][:], 1.0 / H)
                        ve.tensor_tensor(tmp[:], mean[:], mean[:], op=OP.mult)
                        nc.vector.scalar_tensor_tensor(
                            varh[:], ctx["ps2"][:], 1.0 / H, tmp[:],
                            op0=OP.mult, op1=OP.subtract)
                        nc.scalar.activation(sd[:], varh[:], AF.Sqrt,
                                             bias=cc["cols_s"][0:1, 8:9])
                        nc.vector.reciprocal(rstd[:], sd[:])
                    ctx["mean"], ctx["rstd"] = mean, rstd

                def s3():
                    pmb = ptile([H, W], f"pmb{nm}")
                    prb = ptile([H, W], f"prb{nm}")
                    if fin:
                        nc.tensor.matmul(pmb[:], cc["rows32_s"][:, 0:H],
                                         ctx["mean"][:])
                        nc.tensor.matmul(
                            prb[:], cc["rows32_s"][:, gidx * H:(gidx + 1) * H],
                            ctx["rstd"][:])
                    else:
                        nc.tensor.matmul(pmb[:], cc["onesr_s"][:], ctx["mean"][:])
                        nc.tensor.matmul(prb[:], gamma_row[:], ctx["rstd"][:])
                    ctx["pmb"], ctx["prb"] = pmb, prb

                return ctx, [s1, s2, s3]

            def tail_parts(g, lo=0, W=GROUP, last_piece=True, out=None,
                           use_php=False, eng=None, f32stats=None):
                gs = gstate[g]
                n0 = g * GROUP + lo
                nsl = slice(lo, lo + W)
                fin = use_php if f32stats is None else f32stats
                ve = eng if eng is not None else nc.vector
                alt = nc.gpsimd if ve is nc.vector else nc.vector
                def ptile(shape, name):
                    if use_php:
                        return php.tile(shape, F32, tag="ph", name=name)
                    return pgr.tile(shape, F32, tag="gp", name=name)
                ctx = {}

                def p1(kr=None):  # K-sum folded into W3 (PE)
                    if kr is None:
                        kr = range(K)
                    if "pdh" not in ctx:
                        ctx["pdh"] = pgr.tile([H, W], F32, tag="gp", name="pdh")
                        ctx["g2k"] = (gs["g2"][:]
                                      .rearrange("p (n k) -> p k n", k=K)[:, :, nsl])
                    pdh, g2k = ctx["pdh"], ctx["g2k"]
                    if ones_masks:
                        # pairs of neighbors per DoubleRow matmul: both weight
                        # tiles are w3, moving [H, 2, W] = (g2[k], g2[k+1])
                        for k in range(0, K, 2):
                            nc.tensor.matmul(
                                pdh[:], cc["w3d_s"][:],
                                g2k[:, k:k + 2, :],
                                start=(k == 0), stop=(k == K - 2),
                                perf_mode=PM.DoubleRow)
                    else:
                        last = b3z
                        for k in kr:
                            nc.tensor.matmul(
                                pdh[:], cc["w3b_s"][:], g2k[:, k, :],
                                start=(k == 0), stop=(last and k == K - 1))
                        if kr[-1] == K - 1 and not b3z:
                            nc.tensor.matmul(pdh[:], cc["b3s_s"][:],
                                             gs["msum"][:, nsl],
                                             start=False, stop=True)
                    ctx["pdh"] = pdh

                def p2():  # residual add + bf16/sq copies for LN1 stats
                    x = grpp.tile([H, W], F32, tag="x", name="x")
                    pscale = 1.0 / 256.0 if ones_masks else 1.0
                    nc.vector.scalar_tensor_tensor(
                        x[:], ctx["pdh"][:], pscale, gs["hvf"][:, nsl],
                        op0=OP.mult, op1=OP.add)
                    sq_dt = F32 if fin else BF16
                    if fin:
                        xb = x       # f32 stats read the residual directly
                    else:
                        xb = grpp.tile([H, W], BF16, tag="xb", name="xb")
                        ve.tensor_copy(xb[:], x[:])
                    sqx = grpp.tile([H, W], sq_dt, tag="sqx", name="sqx")
                    alt.tensor_tensor(sqx[:], x[:], x[:], op=OP.mult)
                    ctx["x"], ctx["xb"], ctx["sqx"] = x, xb, sqx

                ln1, (l1a, l1b, l1c) = None, (None, None, None)
                ln1_parts = {}

                def p3():
                    c, parts = ln_block(lambda: ctx["x"][:],
                                        lambda: ctx["xb"][:],
                                        lambda: ctx["sqx"][:],
                                        cc["g1row_s"], 2, ln1bz, "a", W, ptile, ve,
                                        fin=fin, gidx=1)
                    ln1_parts["ctx"] = c
                    parts[0]()
                    ln1_parts["rest"] = parts[1:]

                def p4():
                    ln1_parts["rest"][0]()   # mean/rstd small ops + sqrt

                def p5():
                    ln1_parts["rest"][1]()   # pmb/prb broadcast matmuls

                def p6():  # ln1 finish -> xln (bf16)
                    c = ln1_parts["ctx"]
                    t1 = grpp.tile([H, W], F32, tag="t1", name="t1")
                    nc.vector.tensor_tensor(t1[:], ctx["x"][:], c["pmb"][:],
                                            op=OP.subtract)
                    xln = grpp.tile([H, W], BF16, tag="xln", name="xln")
                    nc.vector.tensor_tensor(xln[:], t1[:], c["prb"][:], op=OP.mult)
                    if not ln1bz:
                        ve.tensor_scalar(xln[:], xln[:], scalar1=col(2),
                                         scalar2=None, op0=OP.add)
                    ctx["xln"] = xln

                def p7():  # FFN half 1 (PE + ACT)
                    gf = grpp.tile([H, 4, W], BF16, tag="gf", name="gf")
                    ctx["gf"] = gf
                    if W <= 128:
                        pf = ptile([H, 4, W], "pf1")
                        for j in range(4):
                            nc.tensor.matmul(pf[:, j, :],
                                             cc["win_s"][:, 128 * j:128 * (j + 1)],
                                             ctx["xln"][:])
                        if winbz:
                            nc.scalar.activation(gf[:], pf[:], AF.Gelu, bias=0.0)
                        else:
                            for j in range(4):
                                nc.scalar.activation(gf[:, j, :], pf[:, j, :],
                                                     AF.Gelu, bias=col(4 + j))
                        return
                    pf = ptile([H, 2, W], "pf1")
                    for j in range(2):
                        nc.tensor.matmul(pf[:, j, :], cc["win_s"][:, 128 * j:128 * (j + 1)],
                                         ctx["xln"][:])
                    if winbz:
                        nc.scalar.activation(gf[:, 0:2, :], pf[:], AF.Gelu, bias=0.0)
                    else:
                        for j in range(2):
                            nc.scalar.activation(gf[:, j, :], pf[:, j, :], AF.Gelu,
                                                 bias=col(4 + j))

                def p8():  # FFN half 2
                    if W <= 128:
                        return
                    gf = ctx["gf"]
                    pf = ptile([H, 2, W], "pf2")
                    for j in range(2):
                        nc.tensor.matmul(pf[:, j, :],
                                         cc["win_s"][:, 128 * (2 + j):128 * (3 + j)],
                                         ctx["xln"][:])
                    if winbz:
                        nc.scalar.activation(gf[:, 2:4, :], pf[:], AF.Gelu, bias=0.0)
                    else:
                        for j in range(2):
                            nc.scalar.activation(gf[:, 2 + j, :], pf[:, j, :], AF.Gelu,
                                                 bias=col(6 + j))

                def p9():  # Wout + residual -> z, bf16/sq copies for LN2
                    py = ptile([H, W], "py")
                    for j in range(4):
                        nc.tensor.matmul(py[:], cc["wout_s"][:, 128 * j:128 * (j + 1)],
                                         ctx["gf"][:, j, :], start=(j == 0),
                                         stop=(woutbz and j == 3))
                    if not woutbz:
                        nc.tensor.matmul(py[:], cc["woutb_s"][:],
                                         cc["onesg_s"][:, :W],
                                         start=False, stop=True)
                    z = grpp.tile([H, W], F32, tag="z", name="z")
                    nc.vector.tensor_tensor(z[:], ctx["xln"][:], py[:], op=OP.add)
                    if fin:
                        zb = z
                    else:
                        zb = grpp.tile([H, W], BF16, tag="zb", name="zb")
                        ve.tensor_copy(zb[:], z[:])
                    sqz = grpp.tile([H, W], F32 if fin else BF16, tag="sqz",
                                    name="sqz")
                    alt.tensor_tensor(sqz[:], z[:], z[:], op=OP.mult)
                    ctx["z"], ctx["zb"], ctx["sqz"] = z, zb, sqz

                ln2_parts = {}

                def p10():
                    c, parts = ln_block(lambda: ctx["zb"][:],
                                        lambda: ctx["zb"][:],
                                        lambda: ctx["sqz"][:],
                                        cc["g2row_s"], 3, ln2bz, "b", W, ptile, ve,
                                        fin=fin, gidx=2)
                    ln2_parts["ctx"] = c
                    parts[0]()
                    ln2_parts["rest"] = parts[1:]

                def p11():
                    ln2_parts["rest"][0]()
                    ln2_parts["rest"][1]()

                def p12():  # ln2 finish + (mask_V) + store
                    c = ln2_parts["ctx"]
                    t1b = grpp.tile([H, W], F32, tag="t1b", name="t1b")
                    nc.vector.tensor_tensor(t1b[:], ctx["z"][:], c["pmb"][:],
                                            op=OP.subtract)
                    if out is not None:
                        # write into a shared output tile; one store covers
                        # all pieces sharing it (issued by the designated one)
                        ot, off, do_store, span = out
                        dst = ot[:, off:off + W]
                        nc.vector.tensor_tensor(dst, t1b[:], c["prb"][:],
                                                op=OP.mult)
                        if not ln2bz:
                            nc.vector.tensor_scalar(dst, dst, scalar1=col(3),
                                                    scalar2=None, op0=OP.add)
                        if do_store:
                            nc.sync.dma_start(outt[:, n0 - off:n0 - off + span],
                                              ot[:])
                        if last_piece:
                            gstate.pop(g)
                        return
                    outf = grpp.tile([H, W], F32, tag="outf", name="outf")
                    nc.vector.tensor_tensor(outf[:], t1b[:], c["prb"][:],
                                            op=OP.mult)
                    if not ln2bz:
                        nc.vector.tensor_scalar(outf[:], outf[:], scalar1=col(3),
                                                scalar2=None, op0=OP.add)
                    if not ones_masks:
                        pmv = pgr.tile([H, W], F32, tag="gp", name="pmv")
                        nc.tensor.matmul(pmv[:], cc["onesr_s"][:], gs["mV"][:, nsl])
                        outm = grpp.tile([H, W], F32, tag="outm", name="outm")
                        ve.tensor_tensor(outm[:], outf[:], pmv[:], op=OP.mult)
                        outf = outm
                    nc.sync.dma_start(outt[:, n0:n0 + W], outf[:])
                    if last_piece:
                        gstate.pop(g)

                return [p1, p2, p3, p4, p5, p6, p7, p8, p9, p10, p11, p12]

            # ---------------- software-pipelined emission ----------------
            pending = []
            total = n_groups * n_st
            group_loads(0)
            mrow_load(0)
            mrow_load(1)
            cc.update(cold_consts())
            for t in range(total + 1):
                g, s = divmod(t, n_st)
                if t < total:
                    if s == 2 and g + 1 < n_groups:
                        group_loads(g + 1)
                    mrow_load(t + 2)
                    stage_A(t)
                if t >= 1:
                    stage_C(t - 1)
                    stage_D(t - 1)
                if t < total:
                    stage_B(t)
                if t >= 1 and s == 0 and 1 <= g < n_groups:
                    pending.extend(tail_parts(g - 1))
                # last group: staggered quarter tails, each enqueued right
                # after the D() covering its nodes, so the final dependency
                # chains overlap instead of dangling serially at the end
                if t == total - 4:
                    pending.extend(tail_parts(n_groups - 1, 0, 128,
                                              last_piece=False, f32stats=True))
                if t == total:
                    if ones_masks:
                        ot = grpp.tile([H, 128], F32, tag="outf", name="outf2")
                        # the two final chains run on different vector engines
                        # so their elementwise hops don't queue on each other
                        c0 = tail_parts(n_groups - 1, 128, 96, last_piece=False,
                                        out=(ot, 0, False, 128), use_php=True)
                        c1 = tail_parts(n_groups - 1, 224, 32, last_piece=True,
                                        out=(ot, 96, True, 128), use_php=True)
                    else:
                        c0 = tail_parts(n_groups - 1, 128, 64, last_piece=False,
                                        use_php=True)
                        c1 = tail_parts(n_groups - 1, 192, 64, last_piece=True,
                                        use_php=True)
                    pending.extend(x for tup in zip(c0, c1) for x in tup)
                npop = len(pending) if t >= total - n_st // 2 else (1 + t % 2)
                for _ in range(npop):
                    if pending:
                        pending.pop(0)()
            while pending:
                pending.pop(0)()

    return nc


def _prep_core_inputs(h_V, h_E, mask_V, mask_attend, n_pad):
    """Host marshalling for one core's node slice (feature-major layouts)."""
    n = h_V.shape[0]
    hE8 = np.zeros((C, n_pad * K), E4)
    hE8[:, : n * K] = _f8(h_E.reshape(n * K, C)).T
    hv8 = np.zeros((H, n_pad), E4)
    hv8[:, :n] = _f8(h_V).T
    hVt = np.zeros((H, n_pad), np.float32)
    hVt[:, :n] = h_V.T
    maskr = np.zeros((1, n_pad * K), ml_dtypes.bfloat16)
    maskr[:, : n * K] = _bf(mask_attend.reshape(1, n * K))
    msum = np.zeros((1, n_pad), ml_dtypes.bfloat16)
    msum[:, :n] = _bf(mask_attend.sum(axis=1, dtype=np.float32)).reshape(1, n)
    maskVr = np.zeros((1, n_pad), ml_dtypes.bfloat16)
    maskVr[:, :n] = _bf(mask_V).reshape(1, n)
    return {
        "hE8": np.ascontiguousarray(hE8),
        "hv8D": np.ascontiguousarray(hv8),
        "hVt": np.ascontiguousarray(hVt),
        "maskr": maskr,
        "msumb": msum,
        "maskVr": maskVr,
    }


def kernel(h_V, h_E, mask_V, mask_attend,
           W1_w, W1_b, W2_w, W2_b, W3_w, W3_b,
           ln1_g, ln1_b, Win_w, Win_b, Wout_w, Wout_b, ln2_g, ln2_b):
    from concourse.bass_utils import run_bass_kernel_spmd

    _install_bir_fix()

    h_V = np.asarray(h_V, np.float32)
    h_E = np.asarray(h_E, np.float32)
    mask_V = np.asarray(mask_V, np.float32)
    mask_attend = np.asarray(mask_attend, np.float32)

    n_full = h_V.shape[0]
    per = (n_full + N_CORES - 1) // N_CORES          # 1250
    n_pad = ((per + GROUP - 1) // GROUP) * GROUP     # 1280

    w = dict(W1_w=W1_w, W1_b=W1_b, W2_w=W2_w, W2_b=W2_b, W3_w=W3_w, W3_b=W3_b,
             ln1_g=ln1_g, ln1_b=ln1_b, Win_w=Win_w, Win_b=Win_b,
             Wout_w=Wout_w, Wout_b=Wout_b, ln2_g=ln2_g, ln2_b=ln2_b)
    w = {k: np.asarray(v, np.float32) for k, v in w.items()}

    ones_masks = bool(np.all(mask_attend == 1.0) and np.all(mask_V == 1.0))
    nc = build_nc(w, n_pad, ones_masks=ones_masks)

    in_maps = []
    for c in range(N_CORES):
        lo, hi = c * per, min((c + 1) * per, n_full)
        in_maps.append(_prep_core_inputs(
            h_V[lo:hi], h_E[lo:hi], mask_V[lo:hi], mask_attend[lo:hi], n_pad
        ))

    res = run_bass_kernel_spmd(nc, in_maps, core_ids=list(range(N_CORES)))

    out = np.empty((n_full, H), np.float32)
    for c in range(N_CORES):
        lo, hi = c * per, min((c + 1) * per, n_full)
        out[lo:hi] = res.results[c]["outt"].T[: hi - lo]
    return out



# revision 72
# speedup vs baseline: 1.0137x; 1.0137x over previous
"""Trainium2 Bass kernel for nn_DecLayer (GNN message-passing decoder layer).

Strategy
--------
Data-parallel over nodes: 10000 nodes are padded and split 1280 per core
across 8 NeuronCores.  Per core everything is computed in "transposed space"
(feature dim on SBUF partitions, edges/nodes on the free dim).

Key optimizations over the v1 kernel:
  * h_E is pre-quantized to fp8(e4m3) on the host -> HBM traffic for the
    dominant stream drops 4x vs f32 (15.7MB/core, ~44us at modeled BW).
  * All edge-level matmuls run in fp8 DoubleRow perf mode (2 contraction
    tiles per pass, 0.5 cycles/row).  Weights are pre-scaled by powers of
    two into the fp8 normal range; the activation instruction's `scale`
    operand undoes the scaling for free.  Odd contraction tiles are padded
    by reading a data tile twice against zero weights (stride-0 AP).
  * The K=32 neighbor sum is folded into the W3 matmul: 32 accumulating
    PSUM matmuls with k-strided moving APs replace the (slow, no-perf-mode)
    DVE TensorReduce entirely.
  * LayerNorm stats/broadcasts use bf16 ones-matmuls (1 cycle/row instead
    of 4 for f32).
  * gelu instructions cover 1024 edges (2 PSUM banks) to amortize ACT
    SBUF-access overhead; ScalarE is the bottleneck engine at ~93us busy.
    Individual matmuls stay within one PSUM bank (512 f32) as the ISA
    requires; the 2-bank activation reads span both halves.
  * Group tails (W3/LN/FFN/LN2/store) are split into parts and interleaved
    into the supertile pipeline (2 parts per supertile); the last group's
    tail runs as staggered pieces, the final two half-width chains zipped.
    The latency-exposed final pieces use f32 stats matmuls (no bf16 copy
    hop) and f32 row constants.  PSUM-reading elementwise ops always run
    on DVE (GPSIMD has no PSUM access).
  * The K=32 neighbor sum pairs neighbors k/k+1 per DoubleRow matmul
    (16 matmuls per 256-node piece).
  * Startup: the first 1024-edge h_E chunk + packed W1 go out first on
    the SP/ACT HWDGE queues (one DMA each); cold constants are merged
    into 5 composite DMAs so the SP issue queue stays short; a PE warmup
    matmul chain starts the p-state ramp at t~0.  The SWDGE bulk stream
    opens with a tiny chunk so its second DMA lands after the startup
    critical loads.

Modeled per-core time: ~112.8us (vs 119us for the previous fp8 kernel and
180us for the f32/bf16 v1); measured end-to-end hardware rel-err vs the
f32 reference: 4.7e-3.

If the runtime masks are all ones (true for this problem's setup_inputs),
the kernel compiles without any mask handling; a general path handles
arbitrary masks.
"""

import json

import numpy as np
import ml_dtypes

import concourse.bass as bass
import concourse.mybir as mybir
import concourse.tile as tile

F32 = mybir.dt.float32
BF16 = mybir.dt.bfloat16
FP8 = mybir.dt.float8e4
AF = mybir.ActivationFunctionType
OP = mybir.AluOpType
AX = mybir.AxisListType
PM = mybir.MatmulPerfMode

E4 = ml_dtypes.float8_e4m3

H = 128        # hidden
C = 384        # edge feature dim
K = 32         # neighbors
N_CORES = 8
GROUP = 256                      # nodes per group
ST_NODES = 32                    # nodes per supertile
ST_E = ST_NODES * K              # 1024 edges per supertile
SCALE = 30.0
EPS = 1e-5
S1 = 16.0                        # W1 fp8 pre-scale
S2 = 8.0                         # W2 fp8 pre-scale
S3 = 256.0 / SCALE               # W3 fp8 pre-scale (folds 1/SCALE)


# ---------------------------------------------------------------------------
# walrus workaround: this build rejects >1 sync wait per instruction; split
# extra waits into standalone EventSemaphore instructions on the same engine
# (engines execute their stream in order, so semantics are preserved).
# ---------------------------------------------------------------------------
def _split_multi_waits(bir_json: bytes) -> bytes:
    m = json.loads(bir_json)
    for f in m.get("functions", []):
        for b in f.get("blocks", []):
            out = []
            for inst in b.get("instructions", []):
                si = inst.get("sync_info")
                waits = (si or {}).get("on_wait") or []
                if len(waits) > 1:
                    for j, w in enumerate(waits[:-1]):
                        out.append({
                            "debug": inst.get("debug", 0),
                            "engine": inst["engine"],
                            "ins": [], "outs": [],
                            "name": f"{inst['name']}_wsplit{j}",
                            "opcode": "EventSemaphore",
                            "sync_info": {"on_update": [], "on_wait": [w]},
                        })
                    si["on_wait"] = waits[-1:]
                out.append(inst)
            b["instructions"] = out
    return json.dumps(m).encode()


def _install_bir_fix():
    import concourse.bass_utils as bu
    import concourse.bass2jax as b2j
    if getattr(bu, "_wsplit_installed", False):
        return
    orig = bu.compile_bir_kernel

    def patched(bir_json, tmpdir, neff_name="file.neff"):
        return orig(_split_multi_waits(bir_json), tmpdir, neff_name)

    bu.compile_bir_kernel = patched
    b2j.compile_bir_kernel = patched
    bu._wsplit_installed = True


def _bf(x):
    return np.asarray(x, dtype=ml_dtypes.bfloat16)


def _f8(x):
    return np.asarray(x, dtype=E4)


def _dr_pair(a, b):
    """Stack two [128, M] weight blocks into DoubleRow lhsT layout
    [128, 2*M] = [p, (t m)]."""
    return np.stack([a, b], axis=1).reshape(128, -1)


def build_nc(w, n_nodes, ones_masks=False):
    """Build the per-core Bass kernel. `w` holds the (host numpy) weights."""
    assert n_nodes % GROUP == 0
    n_groups = n_nodes // GROUP
    n_edges = n_nodes * K
    n_st = GROUP // ST_NODES

    nc = bass.Bass()

    hE8 = nc.dram_tensor("hE8", [C, n_edges], FP8, kind="ExternalInput")
    hv8D = nc.dram_tensor("hv8D", [H, n_nodes], FP8, kind="ExternalInput")
    hVt = nc.dram_tensor("hVt", [H, n_nodes], F32, kind="ExternalInput")
    maskr = nc.dram_tensor("maskr", [1, n_edges], BF16, kind="ExternalInput")
    msumb = nc.dram_tensor("msumb", [1, n_nodes], BF16, kind="ExternalInput")
    maskVr = nc.dram_tensor("maskVr", [1, n_nodes], BF16, kind="ExternalInput")
    outt = nc.dram_tensor("outt", [H, n_nodes], F32, kind="ExternalOutput")

    # --- inline constants (weights known at build time) ---
    W1 = w["W1_w"]          # [H+C, H]
    zero8 = np.zeros((128, H), E4)
    w1a_h = _dr_pair(_f8(W1[H:H + 128] * S1), _f8(W1[H + 128:H + 256] * S1))
    w1b_h = _dr_pair(_f8(W1[H + 256:H + 384] * S1), zero8)
    w1v_h = _dr_pair(_f8(W1[:H] * S1), zero8)
    w2d_h = _dr_pair(_f8(w["W2_w"] * S2), zero8)
    w3d_h = _dr_pair(_f8(w["W3_w"] * S3), zero8)
    w3b_h = _bf(w["W3_w"] / SCALE)                      # general-mask path
    wout_h = np.concatenate(
        [w["Wout_w"][128 * j: 128 * (j + 1), :] for j in range(4)], axis=1)

    # hot W1 blocks packed into one tensor -> a single startup DMA
    w1all_h = np.concatenate([w1a_h, w1b_h, w1v_h], axis=1)   # [128, 3*256]
    w1allD = nc.inline_tensor(w1all_h, name="w1all")
    if ones_masks:
        # both DoubleRow tiles carry the real weights: the K-sum pairs
        # neighbors k/k+1 per matmul (16 matmuls instead of 32).
        # w2d and w3d are packed into one inline tensor / one DMA.
        w3d_h = _dr_pair(_f8(w["W3_w"] * S3), _f8(w["W3_w"] * S3))
        w23dD = nc.inline_tensor(
            np.concatenate([w2d_h, w3d_h], axis=1), name="w23d")
    else:
        w2dD = nc.inline_tensor(w2d_h, name="w2d")
        w3bD = nc.inline_tensor(w3b_h, name="w3b")
        b3sD = nc.inline_tensor(_bf(w["W3_b"] / SCALE).reshape(1, H), name="b3s")
    # merged cold constants: fewer DMA issues on the SP sequencer
    winoutD = nc.inline_tensor(
        np.concatenate([_bf(w["Win_w"]), _bf(wout_h)], axis=1), name="winout")
    onescD = nc.inline_tensor(np.ones((H, 1), ml_dtypes.bfloat16), name="onesc")
    # row constants [1, X] packed: onesr(H) | onesg(GROUP) | woutb(H) |
    # g1row(H) | g2row(H)
    rows_h = np.concatenate([
        np.ones((1, H), ml_dtypes.bfloat16),
        np.ones((1, GROUP), ml_dtypes.bfloat16),
        _bf(w["Wout_b"]).reshape(1, H),
        _bf(w["ln1_g"]).reshape(1, H),
        _bf(w["ln2_g"]).reshape(1, H),
    ], axis=1)
    rowsD = nc.inline_tensor(rows_h, name="rows")
    # per-partition bias columns: b1, b2, ln1b, ln2b, winb0..3, EPS, ones
    cols_h = np.stack(
        [w["W1_b"], w["W2_b"], w["ln1_b"], w["ln2_b"]]
        + [w["Win_b"][128 * j: 128 * (j + 1)] for j in range(4)]
        + [np.full(H, EPS, np.float32), np.ones(H, np.float32)],
        axis=1,
    ).astype(np.float32)
    colsD = nc.inline_tensor(cols_h, name="cols")
    # f32 row constants for the latency-critical final pieces:
    # [ones; ln1_g; ln2_g]
    rows32_h = np.concatenate(
        [np.ones(H, np.float32), w["ln1_g"].astype(np.float32),
         w["ln2_g"].astype(np.float32)]).reshape(1, 3 * H)
    rows32D = nc.inline_tensor(rows32_h, name="rows32")

    # value specializations (checked against actual weights at build time)
    b1z = not np.any(w["W1_b"])
    b2z = not np.any(w["W2_b"])
    b3z = not np.any(w["W3_b"])
    winbz = not np.any(w["Win_b"])
    woutbz = not np.any(w["Wout_b"])
    ln1bz = not np.any(w["ln1_b"])
    ln2bz = not np.any(w["ln2_b"])

    g2_dt = FP8 if ones_masks else BF16

    with tile.TileContext(nc) as tc:
        with (
            tc.tile_pool(name="const", bufs=1) as constp,
            tc.tile_pool(name="xe", bufs=3) as xep,
            tc.tile_pool(name="g1p", bufs=3) as g1p,
            tc.tile_pool(name="g2p", bufs=3) as g2p,
            tc.tile_pool(name="st", bufs=3) as stp,
            tc.tile_pool(name="grp", bufs=3) as grpp,
            tc.tile_pool(name="ph", bufs=3 if ones_masks else 2,
                         space="PSUM") as php,
            tc.tile_pool(name="pm", bufs=1, space="PSUM") as pmp,
            tc.tile_pool(name="gp", bufs=2, space="PSUM") as pgr,
        ):
            # ---- load constants once (SP/HWDGE; bulk streams go via Pool) ----
            def cload(shape, dt_, src, name):
                t = constp.tile(shape, dt_, name=name)
                nc.sync.dma_start(t[:], src)
                return t

            # PE warm-up: a chain of dummy matmuls keeps the Tensor engine
            # busy from t~0 until the first real W1 data lands (~4.4us), so
            # the p-state ramp reaches full frequency before real work.
            wupb = constp.tile([128, 1], BF16, name="wup")
            nc.vector.memset(wupb[:], 0.0)
            pwu = pgr.tile([1, 512], F32, tag="gp", name="pwu")
            for _ in range(6):
                nc.tensor.matmul(pwu[:], wupb[:],
                                 wupb[:].broadcast_to([128, 512]))

            # hot constants (single packed DMA, emitted from group_loads(0)
            # right after the critical first h_E chunk)
            hot = {}

            def emit_hot_consts():
                # issued from the ACT HWDGE queue (idle until the first gelu)
                # so it doesn't wait behind the first h_E chunk in the SP
                # sequencer
                w1all_s = constp.tile([128, 3, 2, H], FP8, name="w1all_s")
                nc.scalar.dma_start(
                    w1all_s[:],
                    w1allD[:].rearrange("p (j t m) -> p j t m", j=3, t=2))
                hot["w1a"], hot["w1b"], hot["w1v"] = (
                    w1all_s[:, 0], w1all_s[:, 1], w1all_s[:, 2])

            def cold_consts():
                """Emitted after group-0 loads: first used by W2 / the tails.
                Merged into 5 DMAs to keep the SP issue queue short."""
                c = {}
                if ones_masks:
                    w23 = constp.tile([128, 2, 2, H], FP8, name="w23_s")
                    nc.sync.dma_start(
                        w23[:],
                        w23dD[:].rearrange("p (a t m) -> p a t m", a=2, t=2))
                    c["w2d_s"] = w23[:, 0]
                    c["w3d_s"] = w23[:, 1]
                else:
                    c["w2d_s"] = cload(
                        [128, 2, H], FP8,
                        w2dD[:].rearrange("p (t m) -> p t m", t=2), "w2d_s")
                    c["w3b_s"] = cload([128, H], BF16, w3bD[:], "w3b_s")
                    c["b3s_s"] = cload([1, H], BF16, b3sD[:], "b3s_s")
                wio = constp.tile([H, 2, 512], BF16, name="winout_s")
                nc.sync.dma_start(
                    wio[:], winoutD[:].rearrange("p (a m) -> p a m", a=2))
                c["win_s"] = wio[:, 0]
                c["wout_s"] = wio[:, 1]
                rows = constp.tile([1, rows_h.shape[1]], BF16, name="rows_s")
                nc.sync.dma_start(rows[:], rowsD[:])
                c["onesr_s"] = rows[:, 0:H]
                c["onesg_s"] = rows[:, H:H + GROUP]
                c["woutb_s"] = rows[:, H + GROUP:2 * H + GROUP]
                c["g1row_s"] = rows[:, 2 * H + GROUP:3 * H + GROUP]
                c["g2row_s"] = rows[:, 3 * H + GROUP:4 * H + GROUP]
                c["cols_s"] = cload([H, cols_h.shape[1]], F32, colsD[:], "cols_s")
                c["onesc_s"] = cload([H, 1], BF16, onescD[:], "onesc_s")
                c["rows32_s"] = cload([1, 3 * H], F32, rows32D[:], "rows32_s")
                return c

            cc = {}

            def col(i):
                return cc["cols_s"][:, i:i + 1]

            # ---------------- per-group state ----------------
            gstate = {}   # g -> dict of tiles
            ststate = {}  # t -> dict of tiles
            mstate = {}

            def group_loads(g):
                e0 = g * GROUP * K
                n0 = g * GROUP
                st = {}
                st["xe"] = xep.tile([H, 3, GROUP * K], FP8, tag="xe", name="xe")
                hEr = hE8[:].rearrange("(j p) e -> p j e", p=128)
                if g == 0:
                    # startup critical path, all on the fast HWDGE/SP queue:
                    # first 1024-edge chunk -> packed W1 -> hv8; the rest of
                    # the group streams via gpsimd SWDGE.
                    emit_hot_consts()
                    nc.sync.dma_start(st["xe"][:, :, 0:1024],
                                      hEr[:, :, e0:e0 + 1024])
                    # tiny first SWDGE chunk: its DMA grab is ~0.2us, and the
                    # next chunk's descriptor-gen delays the Pool stream's
                    # second DMA past the startup critical loads
                    bounds = [1024, 1056, 2048, 3072, 4096, 6144, 8192]
                else:
                    bounds = [0, 2048, 4096, 8192]
                st["hv8"] = grpp.tile([H, GROUP], FP8, tag="hv8", name="hv8")
                nc.sync.dma_start(st["hv8"][:], hv8D[:, n0:n0 + GROUP])
                st["hvf"] = grpp.tile([H, GROUP], F32, tag="hvf", name="hvf")
                nc.sync.dma_start(st["hvf"][:], hVt[:, n0:n0 + GROUP])
                if not ones_masks:
                    st["mV"] = grpp.tile([1, GROUP], BF16, tag="mV", name="mV")
                    nc.sync.dma_start(st["mV"][:], maskVr[:, n0:n0 + GROUP])
                    if not b3z:
                        st["msum"] = grpp.tile([1, GROUP], BF16, tag="msum", name="msum")
                        nc.sync.dma_start(st["msum"][:], msumb[:, n0:n0 + GROUP])
                # bulk h_E stream: chunked via gpsimd SWDGE so the HWDGE/SP
                # queue stays free for small loads and stores.
                for hh in range(len(bounds) - 1):
                    nc.gpsimd.dma_start(
                        st["xe"][:, :, bounds[hh]:bounds[hh + 1]],
                        hEr[:, :, e0 + bounds[hh]:e0 + bounds[hh + 1]],
                    )
                st["g2"] = g2p.tile([H, GROUP * K], g2_dt, tag="g2", name="g2")
                gstate[g] = st

            def mrow_load(t):
                if ones_masks or t >= n_groups * n_st:
                    return
                mrow = stp.tile([1, ST_E], BF16, tag="mrow", name="mrow")
                nc.sync.dma_start(mrow[:], maskr[:, t * ST_E:(t + 1) * ST_E])
                mstate[t] = mrow

            def stage_A(t):
                """W1 matmuls for supertile t -> ph (2 PSUM banks)."""
                g, s = divmod(t, n_st)
                gs = gstate[g]
                c0 = s * ST_E
                st = {}
                ph1 = php.tile([H, ST_E], F32, tag="ph", name="ph1")
                HB = ST_E // 2          # matmul out must stay in one PSUM bank
                HN = ST_NODES // 2
                for h in range(2):
                    dst = ph1[:, h * HB:(h + 1) * HB]
                    e0h = c0 + h * HB
                    nc.tensor.matmul(dst, hot["w1a"], gs["xe"][:, 0:2, e0h:e0h + HB],
                                     start=True, stop=False, perf_mode=PM.DoubleRow)
                    nc.tensor.matmul(
                        dst, hot["w1b"],
                        gs["xe"][:, 2:3, e0h:e0h + HB].broadcast_to([H, 2, HB]),
                        start=False, stop=False, perf_mode=PM.DoubleRow)
                    n0h = s * ST_NODES + h * HN
                    hv_b = (gs["hv8"][:, n0h:n0h + HN]
                            .rearrange("p (o n) -> p o n", o=1)
                            .broadcast_to([H, 2, HN])
                            .broadcast_to([H, 2, HN, K]))
                    nc.tensor.matmul(dst, hot["w1v"], hv_b,
                                     start=False, stop=True, perf_mode=PM.DoubleRow)
                st["ph1"] = ph1
                if not ones_masks:
                    pm = pmp.tile([H, ST_E], F32, tag="pm", name="pm")
                    mrow = mstate.pop(t)
                    for h in range(2):
                        nc.tensor.matmul(pm[:, h * (ST_E // 2):(h + 1) * (ST_E // 2)],
                                         cc["onesr_s"][:],
                                         mrow[:, h * (ST_E // 2):(h + 1) * (ST_E // 2)])
                    st["pm"] = pm
                ststate[t] = st

            def stage_B(t):
                """gelu1 over 1024 edges; un-scales W1 by 1/S1; fp8 out."""
                st = ststate[t]
                g1 = g1p.tile([H, 1, ST_E], FP8, tag="g1", name="g1")
                nc.scalar.activation(g1[:, 0, :], st["ph1"][:], AF.Gelu,
                                     bias=0.0 if b1z else col(0), scale=1.0 / S1)
                st["g1"] = g1

            def stage_C(t):
                """W2 fp8 DoubleRow (dup-tile) -> ph2."""
                st = ststate[t]
                ph2 = php.tile([H, ST_E], F32, tag="ph", name="ph2")
                HB = ST_E // 2
                for h in range(2):
                    nc.tensor.matmul(
                        ph2[:, h * HB:(h + 1) * HB], cc["w2d_s"][:],
                        st["g1"][:, 0:1, h * HB:(h + 1) * HB]
                        .broadcast_to([H, 2, HB]),
                        start=True, stop=True, perf_mode=PM.DoubleRow)
                st["ph2"] = ph2

            def stage_D(t):
                """gelu2 -> slice of the group g2 tile (fp8 or bf16)."""
                g, s = divmod(t, n_st)
                st = ststate.pop(t)
                gs = gstate[g]
                dst = gs["g2"][:, s * ST_E:(s + 1) * ST_E]
                if ones_masks:
                    nc.scalar.activation(dst, st["ph2"][:], AF.Gelu,
                                         bias=0.0 if b2z else col(1), scale=1.0 / S2)
                else:
                    g2t = stp.tile([H, ST_E], BF16, tag="g2t", name="g2t")
                    nc.scalar.activation(g2t[:], st["ph2"][:], AF.Gelu,
                                         bias=0.0 if b2z else col(1), scale=1.0 / S2)
                    nc.vector.tensor_tensor(dst, g2t[:], st["pm"][:], op=OP.mult)

            # ---------------- group tails ----------------
            def ln_block(src_f32, src_bf, sq_bf, gamma_row, beta_i, beta_z, nm, W,
                         ptile, ve, fin=False, gidx=1):
                """Shared LN piece: stats matmuls -> mean/rstd -> normalized.

                src_f32: [H, GROUP] f32 (the residual input)
                src_bf/sq_bf: bf16 copies (src, src^2) for the stats matmuls
                fin: latency-critical final piece — f32 stats matmuls read the
                residual directly (no bf16 copy hop); gidx selects the f32
                gamma row (1=ln1, 2=ln2).
                Returns list of part-functions; leaves result tile in ctx[nm].
                """
                ctx = {}
                stat_dt = F32 if fin else BF16

                def s1():
                    onesc = (cc["cols_s"][:, 9:10] if fin
                             else cc["onesc_s"][:])
                    ps1 = pgr.tile([1, W], F32, tag="gp", name=f"ps1{nm}")
                    nc.tensor.matmul(ps1[:], onesc, src_bf())
                    ps2 = pgr.tile([1, W], F32, tag="gp", name=f"ps2{nm}")
                    nc.tensor.matmul(ps2[:], onesc, sq_bf())
                    ctx["ps1"], ctx["ps2"] = ps1, ps2

                def s2():
                    mean = grpp.tile([1, W], stat_dt, tag=f"mean{nm}", name=f"mean{nm}")
                    tmp = grpp.tile([1, W], F32, tag=f"m2{nm}", name=f"m2{nm}")
                    varh = grpp.tile([1, W], F32, tag=f"var{nm}", name=f"var{nm}")
                    sd = grpp.tile([1, W], F32, tag=f"sd{nm}", name=f"sd{nm}")
                    rstd = grpp.tile([1, W], stat_dt, tag=f"rstd{nm}", name=f"rstd{nm}")
                    with nc.allow_low_precision("bf16 LN stats validated vs ref"):
                        # PSUM operands must stay off GPSIMD (no PSUM access)
                        nc.vector.tensor_scalar_mul(mean[:], ctx["ps1"][:], 1.0 / H)
                        ve.tensor_tensor(tmp[:], mean[:], mean[:], op=OP.mult)
                        nc.vector.scalar_tensor_tensor(
                            varh[:], ctx["ps2"][:], 1.0 / H, tmp[:],
                            op0=OP.mult, op1=OP.subtract)
                        nc.scalar.activation(sd[:], varh[:], AF.Sqrt,
                                             bias=cc["cols_s"][0:1, 8:9])
                        nc.vector.reciprocal(rstd[:], sd[:])
                    ctx["mean"], ctx["rstd"] = mean, rstd

                def s3():
                    pmb = ptile([H, W], f"pmb{nm}")
                    prb = ptile([H, W], f"prb{nm}")
                    if fin:
                        nc.tensor.matmul(pmb[:], cc["rows32_s"][:, 0:H],
                                         ctx["mean"][:])
                        nc.tensor.matmul(
                            prb[:], cc["rows32_s"][:, gidx * H:(gidx + 1) * H],
                            ctx["rstd"][:])
                    else:
                        nc.tensor.matmul(pmb[:], cc["onesr_s"][:], ctx["mean"][:])
                        nc.tensor.matmul(prb[:], gamma_row[:], ctx["rstd"][:])
                    ctx["pmb"], ctx["prb"] = pmb, prb

                return ctx, [s1, s2, s3]

            def tail_parts(g, lo=0, W=GROUP, last_piece=True, out=None,
                           use_php=False, eng=None, f32stats=None):
                gs = gstate[g]
                n0 = g * GROUP + lo
                nsl = slice(lo, lo + W)
                fin = use_php if f32stats is None else f32stats
                ve = eng if eng is not None else nc.vector
                alt = nc.gpsimd if ve is nc.vector else nc.vector
                def ptile(shape, name):
                    if use_php:
                        return php.tile(shape, F32, tag="ph", name=name)
                    return pgr.tile(shape, F32, tag="gp", name=name)
                ctx = {}

                def p1(kr=None):  # K-sum folded into W3 (PE)
                    if kr is None:
                        kr = range(K)
                    if "pdh" not in ctx:
                        ctx["pdh"] = pgr.tile([H, W], F32, tag="gp", name="pdh")
                        ctx["g2k"] = (gs["g2"][:]
                                      .rearrange("p (n k) -> p k n", k=K)[:, :, nsl])
                    pdh, g2k = ctx["pdh"], ctx["g2k"]
                    if ones_masks:
                        # pairs of neighbors per DoubleRow matmul: both weight
                        # tiles are w3, moving [H, 2, W] = (g2[k], g2[k+1])
                        for k in range(0, K, 2):
                            nc.tensor.matmul(
                                pdh[:], cc["w3d_s"][:],
                                g2k[:, k:k + 2, :],
                                start=(k == 0), stop=(k == K - 2),
                                perf_mode=PM.DoubleRow)
                    else:
                        last = b3z
                        for k in kr:
                            nc.tensor.matmul(
                                pdh[:], cc["w3b_s"][:], g2k[:, k, :],
                                start=(k == 0), stop=(last and k == K - 1))
                        if kr[-1] == K - 1 and not b3z:
                            nc.tensor.matmul(pdh[:], cc["b3s_s"][:],
                                             gs["msum"][:, nsl],
                                             start=False, stop=True)
                    ctx["pdh"] = pdh

                def p2():  # residual add + bf16/sq copies for LN1 stats
                    x = grpp.tile([H, W], F32, tag="x", name="x")
                    pscale = 1.0 / 256.0 if ones_masks else 1.0
                    nc.vector.scalar_tensor_tensor(
                        x[:], ctx["pdh"][:], pscale, gs["hvf"][:, nsl],
                        op0=OP.mult, op1=OP.add)
                    sq_dt = F32 if fin else BF16
                    if fin:
                        xb = x       # f32 stats read the residual directly
                    else:
                        xb = grpp.tile([H, W], BF16, tag="xb", name="xb")
                        ve.tensor_copy(xb[:], x[:])
                    sqx = grpp.tile([H, W], sq_dt, tag="sqx", name="sqx")
                    alt.tensor_tensor(sqx[:], x[:], x[:], op=OP.mult)
                    ctx["x"], ctx["xb"], ctx["sqx"] = x, xb, sqx

                ln1, (l1a, l1b, l1c) = None, (None, None, None)
                ln1_parts = {}

                def p3():
                    c, parts = ln_block(lambda: ctx["x"][:],
                                        lambda: ctx["xb"][:],
                                        lambda: ctx["sqx"][:],
                                        cc["g1row_s"], 2, ln1bz, "a", W, ptile, ve,
                                        fin=fin, gidx=1)
                    ln1_parts["ctx"] = c
                    parts[0]()
                    ln1_parts["rest"] = parts[1:]

                def p4():
                    ln1_parts["rest"][0]()   # mean/rstd small ops + sqrt

                def p5():
                    ln1_parts["rest"][1]()   # pmb/prb broadcast matmuls

                def p6():  # ln1 finish -> xln (bf16)
                    c = ln1_parts["ctx"]
                    t1 = grpp.tile([H, W], F32, tag="t1", name="t1")
                    nc.vector.tensor_tensor(t1[:], ctx["x"][:], c["pmb"][:],
                                            op=OP.subtract)
                    xln = grpp.tile([H, W], BF16, tag="xln", name="xln")
                    nc.vector.tensor_tensor(xln[:], t1[:], c["prb"][:], op=OP.mult)
                    if not ln1bz:
                        ve.tensor_scalar(xln[:], xln[:], scalar1=col(2),
                                         scalar2=None, op0=OP.add)
                    ctx["xln"] = xln

                def p7():  # FFN half 1 (PE + ACT)
                    gf = grpp.tile([H, 4, W], BF16, tag="gf", name="gf")
                    ctx["gf"] = gf
                    if W <= 128:
                        pf = ptile([H, 4, W], "pf1")
                        for j in range(4):
                            nc.tensor.matmul(pf[:, j, :],
                                             cc["win_s"][:, 128 * j:128 * (j + 1)],
                                             ctx["xln"][:])
                        if winbz:
                            nc.scalar.activation(gf[:], pf[:], AF.Gelu, bias=0.0)
                        else:
                            for j in range(4):
                                nc.scalar.activation(gf[:, j, :], pf[:, j, :],
                                                     AF.Gelu, bias=col(4 + j))
                        return
                    pf = ptile([H, 2, W], "pf1")
                    for j in range(2):
                        nc.tensor.matmul(pf[:, j, :], cc["win_s"][:, 128 * j:128 * (j + 1)],
                                         ctx["xln"][:])
                    if winbz:
                        nc.scalar.activation(gf[:, 0:2, :], pf[:], AF.Gelu, bias=0.0)
                    else:
                        for j in range(2):
                            nc.scalar.activation(gf[:, j, :], pf[:, j, :], AF.Gelu,
                                                 bias=col(4 + j))

                def p8():  # FFN half 2
                    if W <= 128:
                        return
                    gf = ctx["gf"]
                    pf = ptile([H, 2, W], "pf2")
                    for j in range(2):
                        nc.tensor.matmul(pf[:, j, :],
                                         cc["win_s"][:, 128 * (2 + j):128 * (3 + j)],
                                         ctx["xln"][:])
                    if winbz:
                        nc.scalar.activation(gf[:, 2:4, :], pf[:], AF.Gelu, bias=0.0)
                    else:
                        for j in range(2):
                            nc.scalar.activation(gf[:, 2 + j, :], pf[:, j, :], AF.Gelu,
                                                 bias=col(6 + j))

                def p9():  # Wout + residual -> z, bf16/sq copies for LN2
                    py = ptile([H, W], "py")
                    for j in range(4):
                        nc.tensor.matmul(py[:], cc["wout_s"][:, 128 * j:128 * (j + 1)],
                                         ctx["gf"][:, j, :], start=(j == 0),
                                         stop=(woutbz and j == 3))
                    if not woutbz:
                        nc.tensor.matmul(py[:], cc["woutb_s"][:],
                                         cc["onesg_s"][:, :W],
                                         start=False, stop=True)
                    z = grpp.tile([H, W], F32, tag="z", name="z")
                    nc.vector.tensor_tensor(z[:], ctx["xln"][:], py[:], op=OP.add)
                    if fin:
                        zb = z
                    else:
                        zb = grpp.tile([H, W], BF16, tag="zb", name="zb")
                        ve.tensor_copy(zb[:], z[:])
                    sqz = grpp.tile([H, W], F32 if fin else BF16, tag="sqz",
                                    name="sqz")
                    alt.tensor_tensor(sqz[:], z[:], z[:], op=OP.mult)
                    ctx["z"], ctx["zb"], ctx["sqz"] = z, zb, sqz

                ln2_parts = {}

                def p10():
                    c, parts = ln_block(lambda: ctx["zb"][:],
                                        lambda: ctx["zb"][:],
                                        lambda: ctx["sqz"][:],
                                        cc["g2row_s"], 3, ln2bz, "b", W, ptile, ve,
                                        fin=fin, gidx=2)
                    ln2_parts["ctx"] = c
                    parts[0]()
                    ln2_parts["rest"] = parts[1:]

                def p11():
                    ln2_parts["rest"][0]()
                    ln2_parts["rest"][1]()

                def p12():  # ln2 finish + (mask_V) + store
                    c = ln2_parts["ctx"]
                    t1b = grpp.tile([H, W], F32, tag="t1b", name="t1b")
                    nc.vector.tensor_tensor(t1b[:], ctx["z"][:], c["pmb"][:],
                                            op=OP.subtract)
                    if out is not None:
                        # write into a shared output tile; one store covers
                        # all pieces sharing it (issued by the designated one)
                        ot, off, do_store, span = out
                        dst = ot[:, off:off + W]
                        nc.vector.tensor_tensor(dst, t1b[:], c["prb"][:],
                                                op=OP.mult)
                        if not ln2bz:
                            nc.vector.tensor_scalar(dst, dst, scalar1=col(3),
                                                    scalar2=None, op0=OP.add)
                        if do_store:
                            nc.sync.dma_start(outt[:, n0 - off:n0 - off + span],
                                              ot[:])
                        if last_piece:
                            gstate.pop(g)
                        return
                    outf = grpp.tile([H, W], F32, tag="outf", name="outf")
                    nc.vector.tensor_tensor(outf[:], t1b[:], c["prb"][:],
                                            op=OP.mult)
                    if not ln2bz:
                        nc.vector.tensor_scalar(outf[:], outf[:], scalar1=col(3),
                                                scalar2=None, op0=OP.add)
                    if not ones_masks:
                        pmv = pgr.tile([H, W], F32, tag="gp", name="pmv")
                        nc.tensor.matmul(pmv[:], cc["onesr_s"][:], gs["mV"][:, nsl])
                        outm = grpp.tile([H, W], F32, tag="outm", name="outm")
                        ve.tensor_tensor(outm[:], outf[:], pmv[:], op=OP.mult)
                        outf = outm
                    nc.sync.dma_start(outt[:, n0:n0 + W], outf[:])
                    if last_piece:
                        gstate.pop(g)

                return [p1, p2, p3, p4, p5, p6, p7, p8, p9, p10, p11, p12]

            # ---------------- software-pipelined emission ----------------
            pending = []
            total = n_groups * n_st
            group_loads(0)
            mrow_load(0)
            mrow_load(1)
            cc.update(cold_consts())
            for t in range(total + 1):
                g, s = divmod(t, n_st)
                if t < total:
                    if s == 2 and g + 1 < n_groups:
                        group_loads(g + 1)
                    mrow_load(t + 2)
                    stage_A(t)
                if t >= 1:
                    stage_C(t - 1)
                    stage_D(t - 1)
                if t < total:
                    stage_B(t)
                if t >= 1 and s == 0 and 1 <= g < n_groups:
                    pending.extend(tail_parts(g - 1))
                # last group: staggered quarter tails, each enqueued right
                # after the D() covering its nodes, so the final dependency
                # chains overlap instead of dangling serially at the end
                if t == total - 4:
                    pending.extend(tail_parts(n_groups - 1, 0, 128,
                                              last_piece=False, f32stats=True))
                if t == total:
                    if ones_masks:
                        ot = grpp.tile([H, 128], F32, tag="outf", name="outf2")
                        # the two final chains run on different vector engines
                        # so their elementwise hops don't queue on each other
                        c0 = tail_parts(n_groups - 1, 128, 96, last_piece=False,
                                        out=(ot, 0, False, 128), use_php=True)
                        c1 = tail_parts(n_groups - 1, 224, 32, last_piece=True,
                                        out=(ot, 96, True, 128), use_php=True)
                    else:
                        c0 = tail_parts(n_groups - 1, 128, 64, last_piece=False,
                                        use_php=True)
                        c1 = tail_parts(n_groups - 1, 192, 64, last_piece=True,
                                        use_php=True)
                    pending.extend(x for tup in zip(c0, c1) for x in tup)
                npop = len(pending) if t >= total - n_st // 2 else 2
                for _ in range(npop):
                    if pending:
                        pending.pop(0)()
            while pending:
                pending.pop(0)()

    return nc


def _prep_core_inputs(h_V, h_E, mask_V, mask_attend, n_pad):
    """Host marshalling for one core's node slice (feature-major layouts)."""
    n = h_V.shape[0]
    hE8 = np.zeros((C, n_pad * K), E4)
    hE8[:, : n * K] = _f8(h_E.reshape(n * K, C)).T
    hv8 = np.zeros((H, n_pad), E4)
    hv8[:, :n] = _f8(h_V).T
    hVt = np.zeros((H, n_pad), np.float32)
    hVt[:, :n] = h_V.T
    maskr = np.zeros((1, n_pad * K), ml_dtypes.bfloat16)
    maskr[:, : n * K] = _bf(mask_attend.reshape(1, n * K))
    msum = np.zeros((1, n_pad), ml_dtypes.bfloat16)
    msum[:, :n] = _bf(mask_attend.sum(axis=1, dtype=np.float32)).reshape(1, n)
    maskVr = np.zeros((1, n_pad), ml_dtypes.bfloat16)
    maskVr[:, :n] = _bf(mask_V).reshape(1, n)
    return {
        "hE8": np.ascontiguousarray(hE8),
        "hv8D": np.ascontiguousarray(hv8),
        "hVt": np.ascontiguousarray(hVt),
        "maskr": maskr,
        "msumb": msum,
        "maskVr": maskVr,
    }


def kernel(h_V, h_E, mask_V, mask_attend,
           W1_w, W1_b, W2_w, W2_b, W3_w, W3_b,
           ln1_g, ln1_b, Win_w, Win_b, Wout_w, Wout_b, ln2_g, ln2_b):
    from concourse.bass_utils import run_bass_kernel_spmd

    _install_bir_fix()

    h_V = np.asarray(h_V, np.float32)
    h_E = np.asarray(h_E, np.float32)
    mask_V = np.asarray(mask_V, np.float32)
    mask_attend = np.asarray(mask_attend, np.float32)

    n_full = h_V.shape[0]
    per = (n_full + N_CORES - 1) // N_CORES          # 1250
    n_pad = ((per + GROUP - 1) // GROUP) * GROUP     # 1280

    w = dict(W1_w=W1_w, W1_b=W1_b, W2_w=W2_w, W2_b=W2_b, W3_w=W3_w, W3_b=W3_b,
             ln1_g=ln1_g, ln1_b=ln1_b, Win_w=Win_w, Win_b=Win_b,
             Wout_w=Wout_w, Wout_b=Wout_b, ln2_g=ln2_g, ln2_b=ln2_b)
    w = {k: np.asarray(v, np.float32) for k, v in w.items()}

    ones_masks = bool(np.all(mask_attend == 1.0) and np.all(mask_V == 1.0))
    nc = build_nc(w, n_pad, ones_masks=ones_masks)

    in_maps = []
    for c in range(N_CORES):
        lo, hi = c * per, min((c + 1) * per, n_full)
        in_maps.append(_prep_core_inputs(
            h_V[lo:hi], h_E[lo:hi], mask_V[lo:hi], mask_attend[lo:hi], n_pad
        ))

    res = run_bass_kernel_spmd(nc, in_maps, core_ids=list(range(N_CORES)))

    out = np.empty((n_full, H), np.float32)
    for c in range(N_CORES):
        lo, hi = c * per, min((c + 1) * per, n_full)
        out[lo:hi] = res.results[c]["outt"].T[: hi - lo]
    return out

